# revision 48
# baseline (speedup 1.0000x reference)
"""Causal self-attention Trainium2 kernel (B=2, T=2048, D=1024, 16 heads).

Sharding: 8 cores = 2 batches x 4 head-groups (4 heads each).
Per core: column-parallel qkv, local attention, row-parallel proj producing a
partial output; host sums the 4 partials per batch and adds proj bias.
"""

import json
import math
import os

import numpy as np
import ml_dtypes

import bass_rust
import concourse.bass as bass
import concourse.bass2jax as bass2jax
import concourse.bass_utils as bass_utils
import concourse.mybir as mybir
import concourse.tile as tile
from concourse.tile import TileContext
from concourse.vector_clock import ScopedClock
from concourse.bass_utils import run_bass_kernel_spmd

BF16 = mybir.dt.bfloat16
F32 = mybir.dt.float32
NP_BF16 = ml_dtypes.bfloat16

# best-known schedule configuration (overridable via real env vars)
_BEST_CFG = {
    "K_SCK_POOL": "010",      # k hi-dup scatter via Pool SWDGE
    "K_ST_ACT_CHUNKS": "1",   # chunk 0 PSUM staging on Act
    "K_START_2Q": "1",        # startup loads on two issue queues
    "K_PROJ_DELAY": "1",      # proj emitted last = PE filler
    "K_NORM_SPLIT3": "4",     # tj3 norm chain split in column quarters
    "K_PV_DEMOTE": "120",     # PV yields priority to scores (feeds Act)
    "K_PROJ_ACT_ALL3": "1",   # tj3 proj copies on Act (idle at tail)
    "K_HALF_PLAN": "1",       # per-head-pair attention priority windows
    "K_SP3_REV": "1",         # tj3 sp order reversed (small pieces last)
    "K_PROJ3_MASK": "023457", # tj3 proj copies: these pieces on Act, rest DVE
    "K_M_ORDER": "0,2,1,3",   # chunk m-block order: q01,k01,q23,k23
    "K_PP_BUFS": "4",         # P-tile pool depth (exp can run 4 ahead of PV)
    "K_QSP_BUFS": "4",        # split staging pool depth
    "K_ZEROS_SCALAR": "1",    # KS pair1 zeros load on the scalar queue
    "K_X0_SPLIT": "1",        # chunk0 X hi load split by c-halves
}
for _k, _v in _BEST_CFG.items():
    os.environ.setdefault(_k, _v)

D_MODEL = 1024
N_HEAD = 16
D_HEAD = 64
B = 2
T = 2048
G = 4                    # head-groups (tensor parallel)
HPC = N_HEAD // G        # heads per core = 4
QKC = 2 * HPC * D_HEAD   # q+k rows per core = 512
VC = HPC * D_HEAD        # v cols per core = 256
TCH = 512                # t-chunk (matmul moving free dim)
NTJ = T // TCH           # 4 t-chunks
NSI = T // 128           # 16 s-blocks
KC = D_MODEL // 128      # 8 contraction chunks over d_model
SCALE = 1.0 / math.sqrt(D_HEAD)


def _split_multi_waits(bir_json: bytes) -> bytes:
    """The walrus build in this container accepts at most one sync-wait
    command per instruction. Split instructions with N>1 waits into N-1
    preceding single-wait NoOps on the same engine (sequential waits AND
    together, so semantics are unchanged)."""
    bir = json.loads(bir_json)
    ctr = 0
    changed = False
    for func in bir.get("functions", []):
        for blk in func.get("blocks", []):
            out = []
            for inst in blk.get("instructions", []):
                si = inst.get("sync_info")
                waits = (si or {}).get("on_wait") or []
                if len(waits) > 1:
                    changed = True
                    for w in waits[:-1]:
                        ctr += 1
                        out.append(
                            {
                                "debug": inst.get("debug", 0),
                                "engine": inst.get("engine"),
                                "ins": [],
                                "name": f"WSPLIT-{ctr}",
                                "opcode": "NoOp",
                                "outs": [],
                                "sync_info": {"on_update": [], "on_wait": [w]},
                            }
                        )
                    si["on_wait"] = [waits[-1]]
                out.append(inst)
            blk["instructions"] = out
    if not changed:
        return bir_json
    return json.dumps(bir).encode()


_orig_compile_bir_kernel = bass_utils.compile_bir_kernel.__wrapped__ if hasattr(
    bass_utils.compile_bir_kernel, "__wrapped__"
) else bass_utils.compile_bir_kernel


def _patched_compile_bir_kernel(bir_json, tmpdir, neff_name="file.neff"):
    return _orig_compile_bir_kernel(_split_multi_waits(bir_json), tmpdir, neff_name)


def _patch_drain():
    """The walrus build in this container rejects >1 sync-wait command per
    instruction. Patch the compile path to split waits, and the TileContext
    terminal drain to emit single-wait SP nops."""
    if getattr(TileContext, "_drain_patched", False):
        return
    bass_utils.compile_bir_kernel = _patched_compile_bir_kernel
    bass2jax.compile_bir_kernel = _patched_compile_bir_kernel

    def _drain_and_barrier(self, tick_clock, wait_clock):
        nc = self.nc
        probe = nc.sync.nop()
        wait_clock.add_sem_waits(
            probe.ins, ScopedClock({None: tick_clock.global_clock})
        )
        si = probe.ins.sync_info
        waits = list(si.on_wait) if si is not None else []
        if si is not None:
            si.on_wait = waits[:1]
            probe.ins.sync_info = si
        for w in waits[1:]:
            n = nc.sync.nop()
            n.ins.sync_info = bass_rust.SyncInfo(on_wait=[w], on_update=[])
        nc.sync.drain()
        nc.all_engine_barrier()
        popped = nc._tile_sem_poison_stack.pop()
        assert popped is self._sem_poison
        nc.clear_and_free_semaphores(list(self.sems.allocated().values()))
        nc.all_engine_barrier()

    TileContext._drain_and_barrier = _drain_and_barrier
    TileContext._drain_patched = True


F8 = mybir.dt.float8e4
NP_F8 = ml_dtypes.float8_e4m3
S_W = 16.0                      # qkv weight prescale (fp8 subnormal avoidance)
SCALE_FP8 = SCALE / (S_W * S_W)  # folded into the exp activation
DR = mybir.MatmulPerfMode.DoubleRow


def _build_fp8():
    """Causal, no-bias path with fp8 DoubleRow matmuls.

    QKV: 3-term hi/lo split (hi*hi + lo*hi + hi*lo), DoubleRow pairs over
    adjacent d_model chunks -> 0.75x the bf16 matmul time.
    Scores: contraction stacked as [hi(64); lo(64)] on partitions.
      stationary KS pair0 = [Khi; Khi], pair1 = [Klo; 0]
      moving    QS = [Qhi; Qlo], broadcast along the pair dim
      -> one DR matmul per s-block = 2x the bf16 rate, 3-term accurate.
    PV / proj stay bf16.
    """
    _patch_drain()
    nc = bass.Bass()

    # hi/lo packed in one dram tensor so one DMA loads both (HWDGE issue
    # cost is a fixed ~625ns per DMA, so fewer+bigger wins)
    xhl_d = nc.dram_tensor("xhl", [2 * D_MODEL, T], F8, kind="ExternalInput")
    # piece-major qkv weights: per partition [q01|k01|q23|k23|v], each piece
    # c-major so a single dense DMA loads one m-block's weights
    WSZ = KC * (QKC + VC)
    whl_d = nc.dram_tensor("whl", [128, 2 * WSZ], F8, kind="ExternalInput")
    wp_d = nc.dram_tensor("wproj", [VC, D_MODEL], BF16, kind="ExternalInput")
    tri_d = nc.dram_tensor("tri", [128, 128], BF16, kind="ExternalInput")
    zeros_d = nc.dram_tensor("zeros", [64, HPC * T], F8, kind="ExternalInput")
    out_bf16 = os.environ.get("K_OUT_BF16", "1") == "1"
    out_d = nc.dram_tensor(
        "out", [T, D_MODEL], BF16 if out_bf16 else F32, kind="ExternalOutput"
    )

    pre3 = os.environ.get("K_PRE3", "0") == "1"

    with TileContext(nc) as tc:
        with (
            tc.tile_pool(name="consts", bufs=1) as consts,
            tc.tile_pool(name="qkp", bufs=1) as qkp,
            tc.tile_pool(name="vp", bufs=1) as vp,
            tc.tile_pool(name="pp", bufs=int(os.environ.get("K_PP_BUFS", "3"))) as pp,
            tc.tile_pool(name="p3p", bufs=1) as p3p,
            tc.tile_pool(name="ap_", bufs=int(os.environ.get("K_AP_BUFS", "4"))) as ap_,
            tc.tile_pool(name="rp", bufs=int(os.environ.get("K_RP_BUFS", "3"))) as rp,
            tc.tile_pool(name="op_", bufs=int(os.environ.get("K_OSB_BUFS", "6"))) as op_,
            tc.tile_pool(name="qsp", bufs=int(os.environ.get("K_QSP_BUFS", "3"))) as qsp,
            tc.tile_pool(name="sa_ps", bufs=int(os.environ.get("K_SA_BUFS", "2")), space="PSUM") as sa_ps_pool,
            tc.tile_pool(name="u_ps", bufs=int(os.environ.get("K_U_BUFS", "2")), space="PSUM") as u_ps_pool,
            tc.tile_pool(name="o_ps", bufs=int(os.environ.get("K_O_BUFS", "2")), space="PSUM") as o_ps_pool,
        ):
            # ---- constants ----
            # per-chunk X tiles: staggered loads must not falsely conflict
            # with earlier chunks' reads (range coarsening on one big tile).
            # hi/lo live in one tile per chunk so a single DMA fills both.
            XX_t = [
                consts.tile([128, 2, KC, TCH], F8, name=f"XX{j}") for j in range(NTJ)
            ]
            Xh_t = [XX_t[j][:, 0] for j in range(NTJ)]
            Xl_t = [XX_t[j][:, 1] for j in range(NTJ)]
            WW = consts.tile([128, 2, WSZ], F8)
            Wh = WW[:, 0]
            Wl = WW[:, 1]
            # piece offsets (in elements per partition): q01,k01,q23,k23,v
            W_OFF = {0: 0, 2: 1024, 1: 2048, 3: 3072, "v": 4096}

            def w_slice(t, m):
                o = W_OFF[m]
                ln = 2048 if m == "v" else 1024
                nn = 256 if m == "v" else 128
                return t[:, o : o + ln].rearrange("p (c n) -> p c n", c=KC)
            WP = consts.tile([128, VC // 128, D_MODEL], BF16)
            tri = consts.tile([128, 128], BF16)
            # QS: per head [Qhi(p0:64); Qlo(p64:128)] over t
            QS = qkp.tile([128, HPC, T], F8)
            # KS: per head, pair0 = [Khi; Khi], pair1 = [Klo; 0]
            KS = qkp.tile([128, HPC, 2, T], F8)
            V = vp.tile([128, NSI, HPC, 2 * D_HEAD], BF16)

            xhl_r = xhl_d.rearrange("(v c p) t -> p v c t", p=128, v=2)
            whl_r = whl_d.rearrange("p (v n) -> p v n", v=2)
            CHUNK_ORDER = tuple(
                int(c) for c in os.environ.get("K_CHUNK_ORDER", "0,1,2,3").split(",")
            )
            # startup loads pipelined in half-KC pieces; first chunk's X
            # quarter goes first so the PE can start ~2.5us in
            q0 = CHUNK_ORDER[0]
            lo0, hi0 = q0 * TCH, (q0 + 1) * TCH
            # startup: m0's weight piece + chunk0 X + zeros. chunk0's X stays
            # hi/lo-split (two DMAs) so the PE's first matmuls wait only on
            # the hi part; optional K_X0_SPLIT splits hi again by c-chunks.
            x0_split = os.environ.get("K_X0_SPLIT", "0") == "1"
            if os.environ.get("K_START_2Q", "0") == "1":
                # overlap the two HWDGE issue pipelines at startup
                nc.sync.dma_start(out=WW[:, :, 0:1024], in_=whl_r[:, :, 0:1024])
                if x0_split:
                    nc.scalar.dma_start(
                        out=XX_t[q0][:, 0, 0:4], in_=xhl_r[:, 0, 0:4, lo0:hi0]
                    )
                    nc.scalar.dma_start(
                        out=XX_t[q0][:, 0, 4:8], in_=xhl_r[:, 0, 4:8, lo0:hi0]
                    )
                else:
                    nc.scalar.dma_start(
                        out=XX_t[q0][:, 0], in_=xhl_r[:, 0, :, lo0:hi0]
                    )
                nc.scalar.dma_start(out=XX_t[q0][:, 1], in_=xhl_r[:, 1, :, lo0:hi0])
            else:
                nc.sync.dma_start(out=WW[:, :, 0:1024], in_=whl_r[:, :, 0:1024])
                nc.sync.dma_start(out=XX_t[q0][:, 0], in_=xhl_r[:, 0, :, lo0:hi0])
                nc.sync.dma_start(out=XX_t[q0][:, 1], in_=xhl_r[:, 1, :, lo0:hi0])

            for o, ln in ((1024, 1024), (2048, 1024), (3072, 1024), (4096, 2048)):
                nc.sync.dma_start(out=WW[:, :, o : o + ln], in_=whl_r[:, :, o : o + ln])
            nc.scalar.dma_start(out=tri, in_=tri_d[:, :])
            zq = nc.scalar if os.environ.get("K_ZEROS_SCALAR", "0") == "1" else nc.sync
            zq.dma_start(
                out=KS[64:128, :, 1, :],
                in_=zeros_d.rearrange("p (h t) -> p h t", h=HPC),
            )

            if os.environ.get("K_ONES_UPFRONT", "0") == "1":
                # single ones-fill for every chunk's V sum-columns while the
                # Pool engine is otherwise idle waiting on startup DMAs
                nc.gpsimd.memset(V[:, :, :, D_HEAD : 2 * D_HEAD], 1.0)

            def load_chunk_x(q, eng=None, split=False):
                eng = eng or nc.sync
                lo, hi = q * TCH, (q + 1) * TCH
                if split:
                    # hi first: the first 6 matmuls of a block need only hi
                    eng.dma_start(out=XX_t[q][:, 0], in_=xhl_r[:, 0, :, lo:hi])
                    eng.dma_start(out=XX_t[q][:, 1], in_=xhl_r[:, 1, :, lo:hi])
                else:
                    eng.dma_start(out=XX_t[q], in_=xhl_r[:, :, :, lo:hi])

            def emit_qkv_chunk(j, parts="all"):
                cols = slice(j * TCH, (j + 1) * TCH)
                Xhj, Xlj = Xh_t[j], Xl_t[j]
                if parts == "all":
                    m_list = tuple(
                        int(c) for c in os.environ.get("K_M_ORDER", "0,2,3,1").split(",")
                    )
                elif parts == "q":
                    m_list = (0, 1)
                elif parts == "a":      # heads 0/1 projections: q01 + k01
                    m_list = (0, 2)
                elif parts == "b":      # heads 2/3 projections: k23 + q23
                    m_list = (3, 1)
                elif parts == "v":      # V blocks only
                    m_list = ()
                else:
                    m_list = (2, 3)
                if parts in ("all", "kv", "v") and os.environ.get(
                    "K_ONES_UPFRONT", "0"
                ) != "1":
                    # ones columns for this chunk's V blocks (softmax-sum trick)
                    nc.gpsimd.memset(
                        V[:, 4 * j : 4 * j + 4, :, D_HEAD : 2 * D_HEAD], 1.0
                    )
                # order q0, k0, q1, k1 so head 0/1 attention unblocks first
                for mi, m in enumerate(m_list):
                    ps = o_ps_pool.tile([128, TCH], F32, tag="ops")
                    nmm = 0
                    for w, x in ((Wh, Xhj), (Wl, Xhj), (Wh, Xlj)):
                        wm = w_slice(w, m)
                        for cp in range(KC // 2):
                            nmm += 1
                            nc.tensor.matmul(
                                ps,
                                wm[:, 2 * cp : 2 * cp + 2, :],
                                x[:, 2 * cp : 2 * cp + 2, :],
                                start=(nmm == 1),
                                stop=(nmm == 12),
                                perf_mode=DR,
                            )
                    if m < 2:
                        heads = (2 * m, 2 * m + 1)
                    else:
                        heads = (2 * (m - 2), 2 * (m - 2) + 1)
                    # one full-width hi/lo split per m-block, then per-head
                    # scatter into the stacked layouts via SBUF-SBUF DMA.
                    if m < 2 and str(j) in os.environ.get("K_Q_DIRECT", ""):
                        st_hi = st_lo = None  # q-direct: no staging needed
                    elif os.environ.get("K_DIRECT_SPLIT", "0") == "1":
                        # cast PSUM f32 -> fp8 hi directly (single rounding),
                        # then lo = ps - hi straight from PSUM. GPSIMD can't
                        # read PSUM so the sub is Act/DVE only.
                        st_hi = qsp.tile([128, TCH], F8, tag="sthi")
                        st_lo = qsp.tile([128, TCH], F8, tag="stlo")
                        if j < int(os.environ.get("K_ST_ACT_CHUNKS", "1")):
                            nc.scalar.activation(
                                out=st_hi, in_=ps,
                                func=mybir.ActivationFunctionType.Identity,
                            )
                        else:
                            nc.vector.tensor_copy(out=st_hi, in_=ps)
                        nc.vector.tensor_tensor(
                            st_lo, ps, st_hi, mybir.AluOpType.subtract
                        )
                    else:
                        # legacy: DVE stages to bf16 SBUF first and Pool
                        # splits from there.
                        st_hi = qsp.tile([128, TCH], F8, tag="sthi")
                        st_lo = qsp.tile([128, TCH], F8, tag="stlo")
                        st_bf = qsp.tile([128, TCH], BF16, tag="stbf")
                        c0_dve = (j == 0 or str(j) in os.environ.get("K_DVE_SPLIT_CHUNKS", "") or (m >= 2 and os.environ.get("K_K_DVE", "0") == "1")) and os.environ.get("K_C0_DVE", "1") == "1"
                        if j < int(os.environ.get("K_ST_ACT_CHUNKS", "1")):
                            nc.scalar.activation(
                                out=st_bf, in_=ps,
                                func=mybir.ActivationFunctionType.Identity,
                            )
                        else:
                            nc.vector.tensor_copy(out=st_bf, in_=ps)
                        if c0_dve:
                            nc.vector.tensor_copy(out=st_hi, in_=st_bf)
                            nc.vector.tensor_tensor(
                                st_lo, st_bf, st_hi, mybir.AluOpType.subtract
                            )
                        else:
                            nc.gpsimd.tensor_copy(out=st_hi, in_=st_bf)
                            nc.gpsimd.tensor_tensor(
                                st_lo, st_bf, st_hi, mybir.AluOpType.subtract
                            )
                    sc_eng = nc.scalar if os.environ.get("K_SCATTER_ENG") == "scalar" else nc.sync
                    sc2 = nc.scalar if (j == 0 and os.environ.get("K_C0_2Q", "0") == "1") else sc_eng
                    # optional SWDGE routing: gpsimd DMAs bypass the shared
                    # HWDGE issue device (625ns/DMA) at the cost of Pool
                    # engine time (994ns + 0.34/desc)
                    scq_pool = os.environ.get(
                        f"K_SCQ_POOL{j}", os.environ.get("K_SCQ_POOL", "0")
                    ) == "1"
                    sck_pool = os.environ.get(
                        f"K_SCK_POOL{j}", os.environ.get("K_SCK_POOL", "0")
                    )
                    q_direct = str(j) in os.environ.get("K_Q_DIRECT", "")
                    qd_hi_act = str(j) in os.environ.get("K_QD_HI_ACT", "")
                    if os.environ.get("K_MERGE_SC", "0") == "1" and not q_direct:
                        # merged scatters: fold the head dim of st_hi/st_lo
                        # (partitions 64u..64u+63 hold head u) into a free
                        # dim so ONE DMA writes both heads' slices
                        h0 = heads[0]
                        hi_r = st_hi.rearrange("(h p) t -> p h t", h=2)
                        lo_r = st_lo.rearrange("(h p) t -> p h t", h=2)
                        if m < 2:
                            sc_eng.dma_start(
                                out=QS[0:64, h0 : h0 + 2, cols], in_=hi_r
                            )
                            sc2.dma_start(
                                out=QS[64:128, h0 : h0 + 2, cols], in_=lo_r
                            )
                        else:
                            kp = (sck_pool + "000")[:3]
                            e1 = nc.gpsimd if kp[0] == "1" else sc_eng
                            e2 = nc.gpsimd if kp[1] == "1" else sc2
                            e3 = nc.gpsimd if kp[2] == "1" else sc_eng
                            e1.dma_start(
                                out=KS[0:64, h0 : h0 + 2, 0, cols], in_=hi_r
                            )
                            e2.dma_start(
                                out=KS[64:128, h0 : h0 + 2, 0, cols], in_=hi_r
                            )
                            e3.dma_start(
                                out=KS[0:64, h0 : h0 + 2, 1, cols], in_=lo_r
                            )
                        continue_scatter = False
                    else:
                        continue_scatter = True
                    for u, hh in enumerate(heads):
                        if not continue_scatter:
                            break
                        pr = slice(64 * u, 64 * u + 64)
                        if m < 2 and q_direct:
                            # no scatter DMA: cast PSUM halves straight into
                            # the stacked QS layout with partition offsets
                            if qd_hi_act:
                                nc.scalar.activation(
                                    out=QS[0:64, hh, cols], in_=ps[pr],
                                    func=mybir.ActivationFunctionType.Identity,
                                )
                            else:
                                nc.vector.tensor_copy(
                                    out=QS[0:64, hh, cols], in_=ps[pr]
                                )
                            nc.vector.tensor_tensor(
                                QS[64:128, hh, cols], ps[pr], QS[0:64, hh, cols],
                                mybir.AluOpType.subtract,
                            )
                        elif m < 2:
                            e1 = nc.gpsimd if scq_pool else sc_eng
                            e2 = nc.gpsimd if scq_pool else sc2
                            e1.dma_start(out=QS[0:64, hh, cols], in_=st_hi[pr])
                            e2.dma_start(out=QS[64:128, hh, cols], in_=st_lo[pr])
                        else:
                            # sck_pool: bitmask chars for the 3 k writes
                            # (hi, hi-dup, lo) -> '1' routes to gpsimd
                            kp = (sck_pool + "000")[:3]
                            e1 = nc.gpsimd if kp[0] == "1" else sc_eng
                            e2 = nc.gpsimd if kp[1] == "1" else sc2
                            e3 = nc.gpsimd if kp[2] == "1" else sc_eng
                            e1.dma_start(out=KS[0:64, hh, 0, cols], in_=st_hi[pr])
                            e2.dma_start(out=KS[64:128, hh, 0, cols], in_=st_hi[pr])
                            e3.dma_start(out=KS[0:64, hh, 1, cols], in_=st_lo[pr])
                if parts in ("q", "a", "b"):
                    return
                for i in range(4 * j, 4 * j + 4):
                    ps = o_ps_pool.tile([128, TCH], F32, tag="ops")
                    il = i - 4 * j
                    nmm = 0
                    for w, x in ((Wh, Xhj), (Wl, Xhj), (Wh, Xlj)):
                        wv = w_slice(w, "v")
                        for cp in range(KC // 2):
                            nmm += 1
                            nc.tensor.matmul(
                                ps[:, :VC],
                                x[:, 2 * cp : 2 * cp + 2, il * 128 : (il + 1) * 128],
                                wv[:, 2 * cp : 2 * cp + 2, :],
                                start=(nmm == 1),
                                stop=(nmm == 12),
                                perf_mode=DR,
                            )
                    if str(j) in os.environ.get("K_V_ACT", ""):
                        nc.scalar.activation(
                            out=V[:, i, :, 0:D_HEAD],
                            in_=ps[:, :VC].rearrange("p (h d) -> p h d", h=HPC),
                            func=mybir.ActivationFunctionType.Identity,
                        )
                    else:
                        nc.vector.tensor_copy(
                            out=V[:, i, :, 0:D_HEAD],
                            in_=ps[:, :VC].rearrange("p (h d) -> p h d", h=HPC),
                        )

            P3 = {}
            A_tiles = {}

            def emit_attention_spmajor(tj, head_list):
                """sp-major over a head pair: h0/h1 alternate per sp so the
                PE can compute one head's scores while Act exps the other's.
                Needs 2 live U accumulators (u_ps bufs) and 2 live P tiles."""
                if tj in A_tiles:
                    A = A_tiles[tj]
                else:
                    A = ap_.tile([128, VC // 128, TCH], BF16)
                    A_tiles[tj] = A
                n_si = 4 * tj + 4
                n_sp = n_si // 2
                sp_list = list(range(n_sp))
                if tj == NTJ - 1 and os.environ.get("K_SP3_REV", "0") == "1":
                    sp_list = sp_list[::-1]
                sp_first, sp_last = sp_list[0], sp_list[-1]
                Us = {h: u_ps_pool.tile([2 * D_HEAD, TCH], F32, tag="u", name="U")
                      for h in head_list}
                Ps = {h: pp.tile([128, NSI, TCH], BF16, tag="p", name="P")
                      for h in head_list}
                pvd = int(os.environ.get("K_PV_DEMOTE", "0"))
                for sp in sp_list:
                    spair = (2 * sp, 2 * sp + 1)
                    for h in head_list:
                        P = Ps[h]
                        s_ps = sa_ps_pool.tile([128, 2, TCH], F32, tag="s", name="s_ps")
                        for u_, si in enumerate(spair):
                            coff = 128 * (si - 4 * tj) if si >= 4 * tj else 0
                            ncols = TCH - coff
                            qs_mov = (
                                QS[:, h, tj * TCH + coff : (tj + 1) * TCH]
                                .unsqueeze(1)
                                .broadcast_to([128, 2, ncols])
                            )
                            nc.tensor.matmul(
                                s_ps[:, u_, coff:TCH],
                                KS[:, h, :, si * 128 : (si + 1) * 128],
                                qs_mov,
                                start=True,
                                stop=True,
                                perf_mode=DR,
                            )
                        pcoff = 128 * (spair[0] - 4 * tj) if spair[0] >= 4 * tj else 0
                        nc.scalar.activation(
                            out=P[:, 2 * sp : 2 * sp + 2, pcoff:],
                            in_=s_ps[:, :, pcoff:],
                            func=mybir.ActivationFunctionType.Exp,
                            scale=SCALE_FP8,
                        )
                        for si in spair:
                            if si >= 4 * tj:
                                coff = 128 * (si - 4 * tj)
                                nc.vector.tensor_tensor(
                                    P[:, si, coff : coff + 128],
                                    P[:, si, coff : coff + 128],
                                    tri,
                                    mybir.AluOpType.mult,
                                )
                        with tc.high_priority(offset=-pvd):
                            for si in spair:
                                coff = 128 * (si - 4 * tj) if si >= 4 * tj else 0
                                nc.tensor.matmul(
                                    Us[h][:, coff:TCH],
                                    V[:, si, h, :],
                                    P[:, si, coff:TCH],
                                    start=(sp == sp_first and si == spair[0]),
                                    stop=(sp == sp_last and si == spair[1]),
                                    skip_group_check=True,
                                )
                for h in head_list:
                    pb = 64 * (h % 2)
                    Rb = rp.tile([64, TCH], F32, tag="rbsb")
                    a_slice = A[pb : pb + 64, h // 2, :]
                    nsp_ = int(os.environ.get("K_NORM_SPLIT", "1"))
                    if tj == NTJ - 1:
                        nsp_ = int(os.environ.get("K_NORM_SPLIT3", str(nsp_)))
                    hw_ = TCH // nsp_
                    nrmd = int(os.environ.get("K_NORM_DEMOTE", "0"))
                    with tc.high_priority(offset=-nrmd):
                        for half in range(nsp_):
                            cs = slice(half * hw_, (half + 1) * hw_)
                            nc.vector.reciprocal(
                                Rb[:, cs], Us[h][D_HEAD : 2 * D_HEAD, cs]
                            )
                            nc.vector.tensor_tensor(
                                a_slice[:, cs], Us[h][0:D_HEAD, cs], Rb[:, cs],
                                mybir.AluOpType.mult,
                            )

            def emit_attention(tj, phase="full", sp_lo=0, sp_hi=None, head_list=None):
                if (
                    phase == "full"
                    and head_list is not None
                    and len(head_list) == 2
                    and os.environ.get("K_SP_MAJOR", "0") == "1"
                ):
                    return emit_attention_spmajor(tj, head_list)
                if phase != "scores":
                    if tj in A_tiles:
                        A = A_tiles[tj]
                    else:
                        A = ap_.tile([128, VC // 128, TCH], BF16)
                        A_tiles[tj] = A
                n_si = 4 * tj + 4
                if head_list is None:
                    if tj == NTJ - 1 and os.environ.get("K_HEAD_ORDER3", ""):
                        head_list = [
                            int(c) for c in os.environ["K_HEAD_ORDER3"]
                        ]
                    else:
                        head_list = list(range(HPC))
                for h in head_list:
                    if phase != "scores":
                        U = u_ps_pool.tile([2 * D_HEAD, TCH], F32, tag="u", name="U")
                    if phase == "scores":
                        if h not in P3:
                            P3[h] = p3p.tile(
                                [128, NSI, TCH], BF16, tag=f"p3h{h}", name="P3"
                            )
                        P = P3[h]
                    elif phase == "pv":
                        P = P3[h]
                    else:
                        P = pp.tile([128, NSI, TCH], BF16, tag="p", name="P")
                    n_sp = n_si // 2
                    sp_list = list(range(sp_lo, n_sp if sp_hi is None else sp_hi))
                    if tj == NTJ - 1 and os.environ.get("K_SP3_REV", "0") == "1":
                        sp_list = sp_list[::-1]
                    sp_first, sp_last = sp_list[0], sp_list[-1]
                    for sp in sp_list:
                        spair = (2 * sp, 2 * sp + 1)
                        if phase != "pv":
                            s_ps = sa_ps_pool.tile([128, 2, TCH], F32, tag="s", name="s_ps")
                            for u_, si in enumerate(spair):
                                coff = 128 * (si - 4 * tj) if si >= 4 * tj else 0
                                ncols = TCH - coff
                                qs_mov = (
                                    QS[:, h, tj * TCH + coff : (tj + 1) * TCH]
                                    .unsqueeze(1)
                                    .broadcast_to([128, 2, ncols])
                                )
                                nc.tensor.matmul(
                                    s_ps[:, u_, coff:TCH],
                                    KS[:, h, :, si * 128 : (si + 1) * 128],
                                    qs_mov,
                                    start=True,
                                    stop=True,
                                    perf_mode=DR,
                                )
                            pcoff = 128 * (spair[0] - 4 * tj) if spair[0] >= 4 * tj else 0
                            nc.scalar.activation(
                                out=P[:, 2 * sp : 2 * sp + 2, pcoff:],
                                in_=s_ps[:, :, pcoff:],
                                func=mybir.ActivationFunctionType.Exp,
                                scale=SCALE_FP8,
                            )
                            for si in spair:
                                if si >= 4 * tj:
                                    coff = 128 * (si - 4 * tj)
                                    tri_mode = os.environ.get("K_TRI_ENG", "dve")
                                    if tri_mode == "pool":
                                        tri_eng = nc.gpsimd
                                    elif tri_mode == "split":
                                        tri_eng = (
                                            nc.gpsimd
                                            if tj >= int(os.environ.get("K_TRI_TJ", "2"))
                                            else nc.vector
                                        )
                                    else:
                                        tri_eng = nc.vector
                                    tri_eng.tensor_tensor(
                                        P[:, si, coff : coff + 128],
                                        P[:, si, coff : coff + 128],
                                        tri,
                                        mybir.AluOpType.mult,
                                    )
                        if phase != "scores":
                            # PV never feeds Act -- optionally demote it so
                            # the scheduler prefers scores (which do)
                            pvd = int(os.environ.get("K_PV_DEMOTE", "0"))
                            pv_cs = (
                                tj == NTJ - 1
                                and os.environ.get("K_PV_CS", "0") == "1"
                            )
                            with tc.high_priority(offset=-pvd):
                                for si in spair:
                                    coff = 128 * (si - 4 * tj) if si >= 4 * tj else 0
                                    if pv_cs:
                                        # column-split accumulation: left half
                                        # (cols<256) stops at si n_si-3 so its
                                        # norm/proj run before the last exps
                                        HB = TCH // 2
                                        if coff < HB:
                                            nc.tensor.matmul(
                                                U[:, coff:HB],
                                                V[:, si, h, :],
                                                P[:, si, coff:HB],
                                                start=(sp == sp_first and si == spair[0]),
                                                stop=(si == n_si - 3),
                                                skip_group_check=True,
                                            )
                                        cb = max(coff, HB)
                                        nc.tensor.matmul(
                                            U[:, cb:TCH],
                                            V[:, si, h, :],
                                            P[:, si, cb:TCH],
                                            start=(sp == sp_first and si == spair[0]),
                                            stop=(sp == sp_last and si == spair[1]),
                                            skip_group_check=True,
                                        )
                                    else:
                                        nc.tensor.matmul(
                                            U[:, coff:TCH],
                                            V[:, si, h, :],
                                            P[:, si, coff:TCH],
                                            start=(sp == sp_first and si == spair[0]),
                                            stop=(sp == sp_last and si == spair[1]),
                                            skip_group_check=True,
                                        )
                    if phase != "scores":
                        pb = 64 * (h % 2)
                        Rb = rp.tile([64, TCH], F32, tag="rbsb")
                        a_slice = A[pb : pb + 64, h // 2, :]
                        nsp = int(os.environ.get("K_NORM_SPLIT", "1"))
                        if tj == NTJ - 1:
                            nsp = int(os.environ.get("K_NORM_SPLIT3", str(nsp)))
                        # column-split the recip->mult chain so proj pieces
                        # unblock as halves complete (pipelines the tail)
                        hw_ = TCH // nsp
                        nrmd = int(os.environ.get("K_NORM_DEMOTE", "0"))
                        with tc.high_priority(offset=-nrmd):
                            for half in range(nsp):
                                cs = slice(half * hw_, (half + 1) * hw_)
                                nc.vector.reciprocal(
                                    Rb[:, cs], U[D_HEAD : 2 * D_HEAD, cs]
                                )
                                nc.vector.tensor_tensor(
                                    a_slice[:, cs], U[0:D_HEAD, cs], Rb[:, cs],
                                    mybir.AluOpType.mult,
                                )
                if phase == "scores":
                    return
                if os.environ.get("K_PROJ_DELAY", "0") == "0":
                    emit_proj(tj)

            def emit_proj(tj):
                A = A_tiles[tj]
                proj_split = os.environ.get("K_PROJ_SPLIT", "0") == "1" or (
                    tj >= int(os.environ.get("K_PROJ_SPLIT_TJ", "99"))
                )
                for tb in range(TCH // 128):
                    o_sb = op_.tile([128, D_MODEL], BF16 if out_bf16 else F32)
                    o_tiles = {}
                    if proj_split:
                        for n in range(D_MODEL // TCH):
                            o_tiles[n] = o_ps_pool.tile([128, TCH], F32, tag="ops", name="o_ps")
                            nc.tensor.matmul(
                                o_tiles[n],
                                A[:, 0, tb * 128 : (tb + 1) * 128],
                                WP[:, 0, n * TCH : (n + 1) * TCH],
                                start=True, stop=False,
                            )
                    for n in range(D_MODEL // TCH):
                        if proj_split:
                            o_ps = o_tiles[n]
                            nc.tensor.matmul(
                                o_ps,
                                A[:, 1, tb * 128 : (tb + 1) * 128],
                                WP[:, 1, n * TCH : (n + 1) * TCH],
                                start=False, stop=True,
                            )
                        else:
                            o_ps = o_ps_pool.tile([128, TCH], F32, tag="ops")
                            for c in range(VC // 128):
                                nc.tensor.matmul(
                                    o_ps,
                                    A[:, c, tb * 128 : (tb + 1) * 128],
                                    WP[:, c, n * TCH : (n + 1) * TCH],
                                    start=(c == 0),
                                    stop=(c == VC // 128 - 1),
                                )
                        pm = os.environ.get("K_PROJ3_MASK", "")
                        if pm and tj == NTJ - 1:
                            tail_act = str(2 * tb + n) in pm
                        else:
                            tail_act = (
                                os.environ.get("K_PROJ3_ACT", "1") == "1"
                                and tj == NTJ - 1
                                and ((tb + n) % 2 == 1
                                     or os.environ.get("K_PROJ_ACT_ALL3", "0") == "1")
                            )
                        if tail_act or tj < int(os.environ.get("K_PROJ_ACT_TJ", "0")):
                            nc.scalar.activation(
                                out=o_sb[:, n * TCH : (n + 1) * TCH],
                                in_=o_ps,
                                func=mybir.ActivationFunctionType.Identity,
                            )
                        else:
                            nc.vector.tensor_copy(
                                out=o_sb[:, n * TCH : (n + 1) * TCH], in_=o_ps
                            )
                        st_eng = nc.sync
                        st_mode = os.environ.get("K_STORE_ENG", "sync")
                        if st_mode == "scalar":
                            st_eng = nc.scalar
                        elif st_mode == "pool":
                            st_eng = nc.gpsimd
                        elif st_mode == "mix":
                            st_eng = nc.scalar if (tb + n) % 2 == 1 else nc.sync
                        elif st_mode == "poolmix":
                            st_eng = nc.gpsimd if (tb + n) % 2 == 1 else nc.sync
                        if (os.environ.get("K_ST3_SPLIT", "0") == "1"
                                and tj == NTJ - 1 and (tb + n) % 2 == 1):
                            st_eng = nc.scalar
                        # last t-chunk's stores burst at the very end and
                        # serialize on the shared HWDGE (625ns each); route
                        # the listed store indices through Pool SWDGE, a
                        # separate issue device, to overlap the two streams
                        if tj == NTJ - 1 and str(2 * tb + n) in os.environ.get(
                            "K_ST3_POOL", ""
                        ):
                            st_eng = nc.gpsimd
                        big = os.environ.get("K_BIG_STORE", "0") == "1" or (
                            tj == NTJ - 1
                            and os.environ.get("K_BIG_STORE3", "0") == "1"
                        )
                        if big:
                            if n == D_MODEL // TCH - 1:
                                st_eng.dma_start(
                                    out=out_d[
                                        tj * TCH + tb * 128 : tj * TCH + (tb + 1) * 128,
                                        :,
                                    ],
                                    in_=o_sb,
                                )
                        else:
                            st_eng.dma_start(
                                out=out_d[
                                    tj * TCH + tb * 128 : tj * TCH + (tb + 1) * 128,
                                    n * TCH : (n + 1) * TCH,
                                ],
                                in_=o_sb[:, n * TCH : (n + 1) * TCH],
                            )

            load_eng = nc.scalar if os.environ.get("K_LOAD_ENG") == "scalar" else nc.sync
            stagger = os.environ.get("K_STAGGER", "1") == "1"
            if pre3:
                lo_mode = os.environ.get("K_LOAD_ORDER", "312")
                load_order = tuple(int(c) for c in lo_mode)
            else:
                load_order = CHUNK_ORDER[1:]
            up_loads = load_order[:1] if stagger else load_order
            for qi, q in enumerate(up_loads):
                load_chunk_x(q, load_eng)
                if qi == 0 and not stagger:
                    nc.sync.dma_start(
                        out=WP, in_=wp_d.rearrange("(c p) n -> p c n", p=128)
                    )

            t3 = NTJ - 1
            cpo = {}
            if pre3:
                # chunk3's q-blocks hoisted right after chunk0 so tj3's
                # scores+exp pieces spread across every window; the tail is
                # just the diagonal piece + tj3's PV/norm/proj (PE-dense)
                emit_qkv_chunk(0)
                emit_qkv_chunk(t3, parts="q")
                cpo[0] = tc.cur_priority
                for j in range(1, t3):
                    emit_qkv_chunk(j)
                    cpo[j] = tc.cur_priority
                emit_qkv_chunk(t3, parts="kv")
                cpo[t3] = tc.cur_priority
                n_hoist = int(os.environ.get("K_HOIST", "3"))
                for j in range(t3):
                    with tc.high_priority(offset=tc.cur_priority - cpo[j]):
                        emit_attention(j)
                        if j < n_hoist:
                            emit_attention(t3, phase="scores", sp_lo=2 * j, sp_hi=2 * j + 2)
                with tc.high_priority(offset=tc.cur_priority - cpo[t3]):
                    for j in range(n_hoist, t3):
                        emit_attention(t3, phase="scores", sp_lo=2 * j, sp_hi=2 * j + 2)
                    emit_attention(t3, phase="scores", sp_lo=2 * t3, sp_hi=2 * t3 + 2)
                emit_attention(t3, phase="pv")
                if os.environ.get("K_PROJ_DELAY", "0") != "0":
                    for j in range(NTJ):
                        emit_proj(j)
            elif os.environ.get("K_STAG01", "0") == "1":
                # stagger chunks 0/1 only: both chunks' heads-0/1 blocks come
                # before chunk0's h2/h3 blocks, so att1-h01 feeds Act ~4us
                # earlier. X1 is already loaded by the common startup section.
                pa, pb = {}, {}
                plan = [
                    ("a", 0), ("a", 1), ("b", 0), ("v", 0), ("x", 2),
                    ("b", 1), ("v", 1), ("a", 2), ("x", 3), ("b", 2),
                    ("v", 2), ("a", 3), ("b", 3), ("v", 3),
                ]
                did_wp = False
                for kind, j in plan:
                    if kind == "x":
                        load_chunk_x(j, load_eng)
                        continue
                    emit_qkv_chunk(j, parts=kind)
                    if kind == "a":
                        pa[j] = tc.cur_priority
                        if not did_wp:
                            nc.sync.dma_start(
                                out=WP,
                                in_=wp_d.rearrange("(c p) n -> p c n", p=128),
                            )
                            did_wp = True
                    elif kind == "b":
                        pb[j] = tc.cur_priority
                for j in range(NTJ):
                    with tc.high_priority(offset=tc.cur_priority - pa[j]):
                        emit_attention(j, head_list=[0, 1])
                    with tc.high_priority(offset=tc.cur_priority - pb[j]):
                        emit_attention(j, head_list=[2, 3])
                if os.environ.get("K_PROJ_DELAY", "0") != "0":
                    for j in range(NTJ):
                        emit_proj(j)
            elif os.environ.get("K_STAG_PLAN", "0") == "1":
                # one-chunk-staggered halves: chunk j+1's heads-0/1 blocks are
                # emitted right after chunk j's, ahead of chunk j's h2/h3
                # blocks, so Act's h01 attention stream arrives ~one half-
                # chunk earlier across the board
                pa, pb = {}, {}
                plan = [
                    ("a", 0), ("x", 1), ("a", 1), ("x", 2), ("b", 0),
                    ("v", 0), ("a", 2), ("x", 3), ("b", 1), ("v", 1),
                    ("a", 3), ("b", 2), ("v", 2), ("b", 3), ("v", 3),
                ]
                did_wp = False
                for step in plan:
                    kind, j = step
                    if kind == "x":
                        load_chunk_x(j, load_eng, split=(j == 1))
                        continue
                    emit_qkv_chunk(j, parts=kind)
                    if kind == "a":
                        pa[j] = tc.cur_priority
                        if not did_wp:
                            nc.sync.dma_start(
                                out=WP,
                                in_=wp_d.rearrange("(c p) n -> p c n", p=128),
                            )
                            did_wp = True
                    elif kind == "b":
                        pb[j] = tc.cur_priority
                for j in range(NTJ):
                    with tc.high_priority(offset=tc.cur_priority - pa[j]):
                        emit_attention(j, head_list=[0, 1])
                    with tc.high_priority(offset=tc.cur_priority - pb[j]):
                        emit_attention(j, head_list=[2, 3])
                if os.environ.get("K_PROJ_DELAY", "0") != "0":
                    for j in range(NTJ):
                        emit_proj(j)
            elif os.environ.get("K_A_FIRST", "0") == "1":
                # all chunks' heads-0/1 qkv blocks first: Act gets a
                # continuous stream of h0/h1 attention across every t-chunk
                # early, then h2/h3 follows. V blocks ride between (PV is
                # demoted filler; pp bufs bound how far exp runs ahead).
                pa, pb = {}, {}
                vplace = os.environ.get("K_V_PLACE", "mid")
                for ji, j in enumerate(CHUNK_ORDER):
                    emit_qkv_chunk(j, parts="a")
                    pa[j] = tc.cur_priority
                    if ji + 1 < len(CHUNK_ORDER):
                        load_chunk_x(CHUNK_ORDER[ji + 1], load_eng)
                    if ji == 0:
                        nc.sync.dma_start(
                            out=WP, in_=wp_d.rearrange("(c p) n -> p c n", p=128)
                        )
                for ji, j in enumerate(CHUNK_ORDER):
                    if vplace == "mid":
                        emit_qkv_chunk(j, parts="v")
                    emit_qkv_chunk(j, parts="b")
                    pb[j] = tc.cur_priority
                    if vplace == "post":
                        emit_qkv_chunk(j, parts="v")
                if vplace == "end":
                    for j in CHUNK_ORDER:
                        emit_qkv_chunk(j, parts="v")
                for j in range(NTJ):
                    with tc.high_priority(offset=tc.cur_priority - pa[j]):
                        emit_attention(j, head_list=[0, 1])
                    with tc.high_priority(offset=tc.cur_priority - pb[j]):
                        emit_attention(j, head_list=[2, 3])
                if os.environ.get("K_PROJ_DELAY", "0") != "0":
                    for j in range(NTJ):
                        emit_proj(j)
            elif os.environ.get("K_HALF_PLAN", "0") == "1":
                # half-granular plan: each chunk's heads-0/1 blocks (q01,k01)
                # unblock that t-chunk's h0/h1 attention before the h2/h3
                # blocks are even emitted, smoothing Act's work arrival
                pa, pb = {}, {}
                for ji, j in enumerate(CHUNK_ORDER):
                    emit_qkv_chunk(j, parts="a")
                    pa[j] = tc.cur_priority
                    emit_qkv_chunk(j, parts="b")
                    emit_qkv_chunk(j, parts="v")
                    pb[j] = tc.cur_priority
                    if stagger and ji + 2 < len(CHUNK_ORDER):
                        load_chunk_x(CHUNK_ORDER[ji + 2], load_eng)
                    if ji == 0 and stagger:
                        nc.sync.dma_start(
                            out=WP, in_=wp_d.rearrange("(c p) n -> p c n", p=128)
                        )
                for j in range(NTJ):
                    with tc.high_priority(offset=tc.cur_priority - pa[j]):
                        emit_attention(j, head_list=[0, 1])
                    with tc.high_priority(offset=tc.cur_priority - pb[j]):
                        emit_attention(j, head_list=[2, 3])
                if os.environ.get("K_PROJ_DELAY", "0") != "0":
                    for j in range(NTJ):
                        emit_proj(j)
            else:
                late = os.environ.get("K_LATE_LOADS", "0") == "1"
                hoist2 = os.environ.get("K_HOIST2", "0") == "1"
                hq = None
                for ji, j in enumerate(CHUNK_ORDER):
                    if hoist2 and j == t3:
                        emit_qkv_chunk(t3, parts="kv")
                    else:
                        emit_qkv_chunk(j)
                    if stagger and not late and ji + 2 < len(CHUNK_ORDER):
                        load_chunk_x(CHUNK_ORDER[ji + 2], load_eng)
                    if stagger and late and 1 <= ji < len(CHUNK_ORDER) - 1:
                        load_chunk_x(CHUNK_ORDER[ji + 1], load_eng)
                    if ji == (1 if late else 0) and stagger:
                        nc.sync.dma_start(
                            out=WP, in_=wp_d.rearrange("(c p) n -> p c n", p=128)
                        )
                    if hoist2 and ji == 1:
                        # chunk3's q-blocks mid-kernel: tj3's early score
                        # pieces can then fill the Act idle before t=40
                        emit_qkv_chunk(t3, parts="q")
                        hq = tc.cur_priority
                    cpo[j] = tc.cur_priority
                pd_mode = os.environ.get("K_PROJ_DELAY", "0")
                if not hoist2:
                    att_prio = {}
                    # K_H3_PRE: heads of the last t-chunk whose scores+exps
                    # are emitted at high priority up front (into P3 tiles),
                    # leaving their PV as pure-PE filler/tail work
                    h3pre = [int(c) for c in os.environ.get("K_H3_PRE", "")]
                    for j in range(NTJ):
                        off = tc.cur_priority - cpo[j]
                        with tc.high_priority(offset=off):
                            if j == NTJ - 1 and h3pre:
                                rest = [h for h in range(HPC) if h not in h3pre]
                                emit_attention(j, phase="scores", head_list=h3pre)
                                emit_attention(j, head_list=rest)
                            else:
                                emit_attention(j)
                            att_prio[j] = tc.cur_priority
                    if h3pre:
                        emit_attention(NTJ - 1, phase="pv", head_list=h3pre)
                else:
                    for j in range(NTJ - 1):
                        with tc.high_priority(offset=tc.cur_priority - cpo[j]):
                            emit_attention(j)
                    with tc.high_priority(offset=tc.cur_priority - hq):
                        emit_attention(t3, phase="scores", sp_lo=0, sp_hi=4)
                    with tc.high_priority(offset=tc.cur_priority - cpo[2]):
                        emit_attention(t3, phase="scores", sp_lo=4, sp_hi=6)
                    with tc.high_priority(offset=tc.cur_priority - cpo[t3]):
                        emit_attention(t3, phase="scores", sp_lo=6, sp_hi=8)
                    emit_attention(t3, phase="pv")
                    att_prio = {j: cpo[j] for j in range(NTJ)}
                if pd_mode == "1":
                    # all proj at the end, base (lowest) priority: the
                    # scheduler uses it as PE filler wherever A is ready
                    for j in range(NTJ):
                        emit_proj(j)
                elif pd_mode == "2":
                    # proj j anchored at attention j+1's priority point
                    for j in range(NTJ):
                        anchor = att_prio.get(min(j + 1, NTJ - 1), None)
                        if j < NTJ - 1 and anchor is not None:
                            with tc.high_priority(offset=tc.cur_priority - anchor):
                                emit_proj(j)
                        else:
                            emit_proj(j)
    return nc


def _build(mask_mode: str, has_qkv_bias: bool, head_pair=None, si_pair=None):
    """mask_mode: 'causal' | 'none' | 'generic'"""
    if head_pair is None:
        head_pair = os.environ.get("K_HEAD_PAIR", "0") == "1"
    if si_pair is None:
        si_pair = os.environ.get("K_SI_PAIR", "1") == "1"
    interleave = os.environ.get("K_INTERLEAVE", "0") == "1"
    _patch_drain()
    nc = bass.Bass()

    xT = nc.dram_tensor("xT", [D_MODEL, T], BF16, kind="ExternalInput")
    wqkv = nc.dram_tensor("wqkv", [D_MODEL, QKC + VC], BF16, kind="ExternalInput")
    wproj = nc.dram_tensor("wproj", [VC, D_MODEL], BF16, kind="ExternalInput")
    if mask_mode == "causal":
        tri_d = nc.dram_tensor("tri", [128, 128], BF16, kind="ExternalInput")
    if mask_mode == "generic":
        maskT_d = nc.dram_tensor("maskT", [T, T], BF16, kind="ExternalInput")
    if has_qkv_bias:
        bqk_d = nc.dram_tensor("bqk", [QKC], F32, kind="ExternalInput")
        bv_d = nc.dram_tensor("bv", [VC], F32, kind="ExternalInput")
    out_d = nc.dram_tensor("out", [T, D_MODEL], F32, kind="ExternalOutput")

    with TileContext(nc) as tc:
        with (
            tc.tile_pool(name="consts", bufs=1) as consts,
            tc.tile_pool(name="qkp", bufs=1) as qkp,
            tc.tile_pool(name="vp", bufs=1) as vp,
            tc.tile_pool(name="pp", bufs=int(os.environ.get("K_PP_BUFS", "3"))) as pp,
            tc.tile_pool(name="p3p", bufs=1) as p3p,
            tc.tile_pool(name="ap_", bufs=int(os.environ.get("K_AP_BUFS", "4"))) as ap_,
            tc.tile_pool(name="rp", bufs=int(os.environ.get("K_RP_BUFS", "3"))) as rp,
            tc.tile_pool(name="op_", bufs=int(os.environ.get("K_OSB_BUFS", "6"))) as op_,
            tc.tile_pool(name="dram_p", bufs=2, space="DRAM") as dram_p,
            tc.tile_pool(name="sa_ps", bufs=int(os.environ.get("K_SA_BUFS", str(4 // (2 if head_pair else 1) // (2 if si_pair else 1)))), space="PSUM") as sa_ps_pool,
            tc.tile_pool(name="sb_ps", bufs=(2 // (2 if si_pair else 1)), space="PSUM") as sb_ps_pool,
            tc.tile_pool(name="u_ps", bufs=int(os.environ.get("K_U_BUFS", "2")), space="PSUM") as u_ps_pool,
            tc.tile_pool(name="o_ps", bufs=int(os.environ.get("K_O_BUFS", "2")), space="PSUM") as o_ps_pool,
        ):
            # ---- load constants ----
            xT_r = xT.rearrange("(c p) t -> p c t", p=128)
            X = consts.tile([128, KC, T], BF16)
            wqkv_r = wqkv.rearrange("(c p) n -> p c n", p=128)
            W = consts.tile([128, KC, QKC + VC], BF16)
            dma_engs = [nc.sync, nc.gpsimd, nc.scalar]
            n_dma_eng = int(os.environ.get("K_DMA_ENGS", "3"))
            NQ = int(os.environ.get("K_XQ", "4"))
            for q in range(NQ):
                lo, hi = q * (T // NQ), (q + 1) * (T // NQ)
                for c in range(KC):
                    if q == 0:
                        dma_engs[c % n_dma_eng].dma_start(
                            out=W[:, c], in_=wqkv_r[:, c]
                        )
                    dma_engs[(q * KC + c + 1) % n_dma_eng].dma_start(
                        out=X[:, c, lo:hi], in_=xT_r[:, c, lo:hi]
                    )
            if mask_mode == "causal":
                tri = consts.tile([128, 128], BF16)
                nc.scalar.dma_start(out=tri, in_=tri_d[:, :])
            WP = consts.tile([128, VC // 128, D_MODEL], BF16)
            nc.sync.dma_start(out=WP, in_=wproj.rearrange("(c p) n -> p c n", p=128))
            if mask_mode == "generic":
                MT = consts.tile([128, NSI, T], BF16)
                nc.sync.dma_start(
                    out=MT, in_=maskT_d.rearrange("(si p) t -> p si t", p=128)
                )
            if has_qkv_bias:
                bqk = consts.tile([128, QKC // 128], F32)
                nc.sync.dma_start(
                    out=bqk, in_=bqk_d.rearrange("(m p) -> p m", p=128)
                )
                bv = consts.tile([128, VC // 128], F32)
                nc.sync.dma_start(out=bv, in_=bv_d.rearrange("(m p) -> p m", p=128))

            # V tile (natural layout). Each head gets 64 ones-columns
            # appended so the PV matmul (M=128, same pass cost as M=65)
            # emits the softmax sums replicated on partitions 64..127 --
            # the reciprocal+normalize then needs no partition broadcast.
            V = vp.tile([128, NSI, HPC, 2 * D_HEAD], BF16)
            nc.vector.memset(V[:, :, :, D_HEAD : 2 * D_HEAD], 1.0)
            QK = qkp.tile([128, QKC // 128, T], BF16)

            def emit_qkv_chunk(j):
                # Q^T / K^T chunk j: [qkrow, t] = sum_c W[c, qkrow] X^T[c, t]
                for m in range(QKC // 128):
                    qk_ps = o_ps_pool.tile([128, TCH], F32, tag="ops")
                    for c in range(KC):
                        nc.tensor.matmul(
                            qk_ps,
                            W[:, c, m * 128 : (m + 1) * 128],
                            X[:, c, j * TCH : (j + 1) * TCH],
                            start=(c == 0),
                            stop=(c == KC - 1),
                        )
                    if has_qkv_bias:
                        nc.scalar.activation(
                            out=QK[:, m, j * TCH : (j + 1) * TCH],
                            in_=qk_ps,
                            func=mybir.ActivationFunctionType.Identity,
                            bias=bqk[:, m : m + 1],
                        )
                    else:
                        nc.vector.tensor_copy(
                            out=QK[:, m, j * TCH : (j + 1) * TCH], in_=qk_ps
                        )
                # V rows for this chunk
                for i in range(4 * j, 4 * j + 4):
                    v_ps = o_ps_pool.tile([128, TCH], F32, tag="ops")
                    for c in range(KC):
                        nc.tensor.matmul(
                            v_ps[:, :VC],
                            X[:, c, i * 128 : (i + 1) * 128],
                            W[:, c, QKC : QKC + VC],
                            start=(c == 0),
                            stop=(c == KC - 1),
                        )
                    nc.vector.tensor_copy(
                        out=V[:, i, :, 0:D_HEAD],
                        in_=v_ps[:, :VC].rearrange("p (h d) -> p h d", h=HPC),
                    )

            pre3 = (
                os.environ.get("K_PRE3", "0") == "1" and mask_mode == "causal"
            )
            P3 = {}

            def emit_attention(tj, phase="full"):
                # phase: "full" | "scores" (S/exp/mask only, into P3 tiles)
                #        | "pv" (PV/norm/proj consuming P3 tiles)
                if phase != "scores":
                    A = ap_.tile([128, VC // 128, TCH], BF16)
                n_si = NSI if mask_mode != "causal" else 4 * tj + 4
                p_slices = 12 if pre3 else NSI
                HGRP = 2 if head_pair else 1
                SGRP = 2 if si_pair else 1
                def emit_head_group(hp):
                    heads = tuple(HGRP * hp + u for u in range(HGRP))
                    Us = {}
                    Ps = {}
                    for h in heads:
                        if phase != "scores":
                            Us[h] = u_ps_pool.tile(
                                [2 * D_HEAD, TCH], F32, tag="u", name="U"
                            )
                        if phase == "scores":
                            P3[h] = p3p.tile(
                                [128, NSI, TCH], BF16, tag=f"p3h{h}", name="P3"
                            )
                            Ps[h] = P3[h]
                        elif phase == "pv":
                            Ps[h] = P3[h]
                        else:
                            Ps[h] = pp.tile(
                                [128, p_slices, TCH], BF16, tag="p", name="P"
                            )
                    sp_order = list(range(n_si // SGRP))
                    if os.environ.get("K_SP_REV", "0") == "1":
                        sp_order = sp_order[::-1]
                    first_sp = sp_order[0]
                    last_sp = sp_order[-1]
                    for sp in sp_order:
                        spair = tuple(SGRP * sp + u for u in range(SGRP))
                        s_tiles = {}
                        for hi, h in enumerate(heads):
                            if phase == "pv":
                                break
                            pool = sa_ps_pool if hi == 0 else sb_ps_pool
                            s_ps = pool.tile([128, SGRP, TCH], F32, tag="s", name="s_ps")
                            s_tiles[h] = s_ps
                            pb = 64 * (h % 2)
                            qm = h // 2
                            km = 2 + h // 2
                            for u, si in enumerate(spair):
                                if mask_mode == "causal" and si >= 4 * tj:
                                    coff = 128 * (si - 4 * tj)
                                else:
                                    coff = 0
                                nc.tensor.matmul(
                                    s_ps[:, u, coff:TCH],
                                    QK[pb : pb + 64, km, si * 128 : (si + 1) * 128],
                                    QK[
                                        pb : pb + 64,
                                        qm,
                                        tj * TCH + coff : (tj + 1) * TCH,
                                    ],
                                    start=True,
                                    stop=True,
                                )
                        exp_split = (
                            os.environ.get("K_EXP_SPLIT", "0") == "1"
                            or tj >= int(os.environ.get("K_EXP_SPLIT_TJ", "99"))
                        )
                        for h in heads:
                            if phase == "pv":
                                break
                            # exp over the si-pair (prefixes of diagonal
                            # blocks hold garbage; never read back)
                            if exp_split:
                                for u in range(SGRP):
                                    nc.scalar.activation(
                                        out=Ps[h][:, SGRP * sp + u, :],
                                        in_=s_tiles[h][:, u, :],
                                        func=mybir.ActivationFunctionType.Exp,
                                        scale=SCALE,
                                    )
                            else:
                                if mask_mode == "causal" and spair[0] >= 4 * tj:
                                    pcoff = 128 * (spair[0] - 4 * tj)
                                else:
                                    pcoff = 0
                                nc.scalar.activation(
                                    out=Ps[h][:, SGRP * sp : SGRP * sp + SGRP, pcoff:],
                                    in_=s_tiles[h][:, :, pcoff:],
                                    func=mybir.ActivationFunctionType.Exp,
                                    scale=SCALE,
                                )
                            for si in spair:
                                if mask_mode == "causal" and si >= 4 * tj:
                                    coff = 128 * (si - 4 * tj)
                                    nc.vector.tensor_tensor(
                                        Ps[h][:, si, coff : coff + 128],
                                        Ps[h][:, si, coff : coff + 128],
                                        tri,
                                        mybir.AluOpType.mult,
                                    )
                            if mask_mode == "generic":
                                for si in spair:
                                    nc.vector.tensor_tensor(
                                        Ps[h][:, si, :],
                                        Ps[h][:, si, :],
                                        MT[:, si, tj * TCH : (tj + 1) * TCH],
                                        mybir.AluOpType.mult,
                                    )
                        for h in heads:
                            if phase == "scores":
                                break
                            for si in spair:
                                if mask_mode == "causal" and si >= 4 * tj:
                                    coff = 128 * (si - 4 * tj)
                                else:
                                    coff = 0
                                nc.tensor.matmul(
                                    Us[h][:, coff:TCH],
                                    V[:, si, h, :],
                                    Ps[h][:, si, coff:TCH],
                                    start=(sp == first_sp and si == spair[0]),
                                    stop=(sp == last_sp and si == spair[-1]),
                                    skip_group_check=True,
                                )
                    for h in heads:
                        if phase == "scores":
                            break
                        # normalize: sums sit replicated on partitions
                        # 64..127 of U; reciprocal them straight to SBUF
                        pb = 64 * (h % 2)
                        Rb_sb = rp.tile([64, TCH], F32, tag="rbsb")
                        nc.vector.reciprocal(Rb_sb, Us[h][D_HEAD : 2 * D_HEAD, :])
                        a_slice = A[pb : pb + 64, h // 2, :]
                        nc.vector.tensor_tensor(
                            a_slice, Us[h][0:D_HEAD, :], Rb_sb, mybir.AluOpType.mult
                        )
                        if has_qkv_bias:
                            nc.scalar.activation(
                                out=a_slice,
                                in_=a_slice,
                                func=mybir.ActivationFunctionType.Identity,
                                bias=bv[pb : pb + 64, h // 2 : h // 2 + 1],
                            )

                head_ilv = os.environ.get("K_HEAD_ILV", "0") == "1"
                for hp in range(HPC // HGRP):
                    if head_ilv and hp % 2 == 1:
                        off = tc.cur_priority - pair_base
                        with tc.high_priority(offset=off):
                            emit_head_group(hp)
                    else:
                        pair_base = tc.cur_priority
                        emit_head_group(hp)
                if phase == "scores":
                    return
                # proj for this t-chunk: out[t, n] = sum_c A^T[c, t] * WP[c, n]
                proj_split = os.environ.get("K_PROJ_SPLIT", "0") == "1"
                for tb in range(TCH // 128):
                    o_sb = op_.tile([128, D_MODEL], BF16 if out_bf16 else F32)
                    o_tiles = {}
                    if proj_split:
                        for n in range(D_MODEL // TCH):
                            o_tiles[n] = o_ps_pool.tile([128, TCH], F32, tag="ops", name="o_ps")
                            nc.tensor.matmul(
                                o_tiles[n],
                                A[:, 0, tb * 128 : (tb + 1) * 128],
                                WP[:, 0, n * TCH : (n + 1) * TCH],
                                start=True, stop=False,
                            )
                    for n in range(D_MODEL // TCH):
                        if proj_split:
                            o_ps = o_tiles[n]
                            nc.tensor.matmul(
                                o_ps,
                                A[:, 1, tb * 128 : (tb + 1) * 128],
                                WP[:, 1, n * TCH : (n + 1) * TCH],
                                start=False, stop=True,
                            )
                        else:
                            o_ps = o_ps_pool.tile([128, TCH], F32, tag="ops")
                            for c in range(VC // 128):
                                nc.tensor.matmul(
                                    o_ps,
                                    A[:, c, tb * 128 : (tb + 1) * 128],
                                    WP[:, c, n * TCH : (n + 1) * TCH],
                                    start=(c == 0),
                                    stop=(c == VC // 128 - 1),
                                )
                        nc.vector.tensor_copy(
                            out=o_sb[:, n * TCH : (n + 1) * TCH], in_=o_ps
                        )
                        st_eng = nc.sync
                        if os.environ.get("K_STORE_ENG", "sync") == "scalar":
                            st_eng = nc.scalar
                        elif os.environ.get("K_STORE_ENG", "sync") == "mix":
                            st_eng = nc.scalar if (tb + n) % 2 == 1 else nc.sync
                        if (os.environ.get("K_ST3_SPLIT", "0") == "1"
                                and tj == NTJ - 1 and (tb + n) % 2 == 1):
                            st_eng = nc.scalar
                        if os.environ.get("K_BIG_STORE", "0") == "1":
                            if n == D_MODEL // TCH - 1:
                                st_eng.dma_start(
                                    out=out_d[
                                        tj * TCH + tb * 128 : tj * TCH + (tb + 1) * 128,
                                        :,
                                    ],
                                    in_=o_sb,
                                )
                        else:
                            st_eng.dma_start(
                                out=out_d[
                                    tj * TCH + tb * 128 : tj * TCH + (tb + 1) * 128,
                                    n * TCH : (n + 1) * TCH,
                                ],
                                in_=o_sb[:, n * TCH : (n + 1) * TCH],
                            )

            prio_mode = os.environ.get("K_PRIO", "1") == "1"
            if interleave:
                for j in range(NTJ):
                    emit_qkv_chunk(j)
                    emit_attention(j)
            elif prio_mode:
                # emit qkv first (program order = dataflow order), but give
                # attention tj a priority window starting right after qkv
                # chunk tj, so the scheduler fills attention stalls with
                # later qkv chunks
                cp = []
                for j in range(NTJ):
                    emit_qkv_chunk(j)
                    cp.append(tc.cur_priority)
                if pre3:
                    # tj3's S/exp/mask precompute as mid-kernel filler
                    # (window right after qkv chunk 3); its PV/norm/proj
                    # run last as a dense pure-PE tail
                    for j in range(NTJ - 1):
                        off = tc.cur_priority - cp[j]
                        with tc.high_priority(offset=off):
                            emit_attention(j)
                    off = tc.cur_priority - cp[NTJ - 1]
                    with tc.high_priority(offset=off):
                        emit_attention(NTJ - 1, phase="scores")
                    emit_attention(NTJ - 1, phase="pv")
                else:
                    for j in range(NTJ):
                        off = tc.cur_priority - cp[j]
                        with tc.high_priority(offset=off):
                            emit_attention(j)
            else:
                for j in range(NTJ):
                    emit_qkv_chunk(j)
                for j in range(NTJ):
                    emit_attention(j)
    return nc


_NC_CACHE: dict = {}


def _use_fp8(mask_mode: str, has_qkv_bias: bool) -> bool:
    if os.environ.get("K_FP8", "1") != "1":
        return False
    return mask_mode == "causal" and not has_qkv_bias


def _get_nc(mask_mode: str, has_qkv_bias: bool):
    if _use_fp8(mask_mode, has_qkv_bias):
        key = "fp8"
        if key not in _NC_CACHE:
            _NC_CACHE[key] = _build_fp8()
        return _NC_CACHE[key]
    key = (mask_mode, has_qkv_bias)
    if key not in _NC_CACHE:
        _NC_CACHE[key] = _build(mask_mode, has_qkv_bias)
    return _NC_CACHE[key]


def classify_inputs(mask, qkv_b):
    m2 = np.asarray(mask).reshape(T, T)
    if np.array_equal(m2 != 0, np.tril(np.ones((T, T), dtype=bool))):
        mask_mode = "causal"
    elif np.all(m2 != 0):
        mask_mode = "none"
    else:
        mask_mode = "generic"
    has_qkv_bias = bool(np.any(np.asarray(qkv_b) != 0.0))
    return mask_mode, has_qkv_bias


def prepare_in_maps(x, mask, qkv_w, qkv_b, proj_w, proj_b):
    x = np.asarray(x, dtype=np.float32)
    qkv_w = np.asarray(qkv_w, dtype=np.float32)
    qkv_b = np.asarray(qkv_b, dtype=np.float32)
    proj_w = np.asarray(proj_w, dtype=np.float32)
    mask_mode, has_qkv_bias = classify_inputs(mask, qkv_b)
    m2 = np.asarray(mask).reshape(T, T)

    tri_np = np.triu(np.ones((128, 128))).astype(NP_BF16)
    in_maps = []
    for b in range(B):
        xT_b = np.ascontiguousarray(x[b].T).astype(NP_BF16)
        for g in range(G):
            qs = qkv_w[:, g * VC : (g + 1) * VC]
            ks = qkv_w[:, D_MODEL + g * VC : D_MODEL + (g + 1) * VC]
            vs = qkv_w[:, 2 * D_MODEL + g * VC : 2 * D_MODEL + (g + 1) * VC]
            im = {
                "xT": xT_b,
                "wqkv": np.ascontiguousarray(
                    np.concatenate([qs, ks, vs], axis=1)
                ).astype(NP_BF16),
                "wproj": np.ascontiguousarray(
                    proj_w[g * VC : (g + 1) * VC, :]
                ).astype(NP_BF16),
            }
            if mask_mode == "causal":
                im["tri"] = tri_np
            if mask_mode == "generic":
                im["maskT"] = np.ascontiguousarray(
                    (m2 != 0).T.astype(NP_BF16)
                )
            if has_qkv_bias:
                im["bqk"] = np.ascontiguousarray(
                    np.concatenate(
                        [qkv_b[g * VC : (g + 1) * VC],
                         qkv_b[D_MODEL + g * VC : D_MODEL + (g + 1) * VC]]
                    )
                ).astype(np.float32)
                im["bv"] = np.ascontiguousarray(
                    qkv_b[2 * D_MODEL + g * VC : 2 * D_MODEL + (g + 1) * VC]
                ).astype(np.float32)
            in_maps.append(im)
    return in_maps


def prepare_in_maps_fp8(x, qkv_w, proj_w):
    x = np.asarray(x, dtype=np.float32)
    qkv_w = np.asarray(qkv_w, dtype=np.float32) * S_W
    proj_w = np.asarray(proj_w, dtype=np.float32) / S_W
    tri_np = np.triu(np.ones((128, 128))).astype(NP_BF16)
    zeros_np = np.zeros((64, HPC * T), dtype=NP_F8)
    in_maps = []
    for b in range(B):
        xT = np.ascontiguousarray(x[b].T)
        xh = xT.astype(NP_F8)
        xl = (xT - xh.astype(np.float32)).astype(NP_F8)
        xhl = np.ascontiguousarray(np.concatenate([xh, xl], axis=0))
        for g in range(G):
            qs = qkv_w[:, g * VC : (g + 1) * VC]
            ks = qkv_w[:, D_MODEL + g * VC : D_MODEL + (g + 1) * VC]
            vs = qkv_w[:, 2 * D_MODEL + g * VC : 2 * D_MODEL + (g + 1) * VC]
            w = np.ascontiguousarray(np.concatenate([qs, ks, vs], axis=1))
            # piece-major layout: per partition [q01|k01|q23|k23|v], each
            # piece c-major (matches the Wh/Wl SBUF tiles)
            pieces = [(0, 128), (256, 384), (128, 256), (384, 512), (512, 768)]
            wp_parts = []
            for a, b in pieces:
                blk = w[:, a:b].reshape(KC, 128, b - a)          # [c, p, n]
                wp_parts.append(blk.transpose(1, 0, 2).reshape(128, -1))
            wpm = np.ascontiguousarray(np.concatenate(wp_parts, axis=1))
            wh = wpm.astype(NP_F8)
            wl = (wpm - wh.astype(np.float32)).astype(NP_F8)
            whl = np.ascontiguousarray(np.concatenate([wh, wl], axis=1))
            in_maps.append(
                {
                    "xhl": xhl,
                    "whl": whl,
                    "wproj": np.ascontiguousarray(
                        proj_w[g * VC : (g + 1) * VC, :]
                    ).astype(NP_BF16),
                    "tri": tri_np,
                    "zeros": zeros_np,
                }
            )
    return in_maps


def kernel(x, mask, qkv_w, qkv_b, proj_w, proj_b):
    proj_b = np.asarray(proj_b, dtype=np.float32)
    mask_mode, has_qkv_bias = classify_inputs(mask, qkv_b)
    nc = _get_nc(mask_mode, has_qkv_bias)
    if _use_fp8(mask_mode, has_qkv_bias):
        in_maps = prepare_in_maps_fp8(x, qkv_w, proj_w)
    else:
        in_maps = prepare_in_maps(x, mask, qkv_w, qkv_b, proj_w, proj_b)

    trace = bool(os.environ.get("KERNEL_TRACE"))
    res = run_bass_kernel_spmd(
        nc, in_maps, core_ids=list(range(B * G)), trace=trace
    )
    globals()["LAST_RESULT"] = res
    outs = [np.asarray(r["out"], dtype=np.float32) for r in res.results]

    final = np.empty((B, T, D_MODEL), dtype=np.float32)
    for b in range(B):
        acc = outs[b * G].copy()
        for g in range(1, G):
            acc += outs[b * G + g]
        final[b] = acc + proj_b[None, :]
    return final

